# revision 2
# baseline (speedup 1.0000x reference)
"""Multi-head attention (2 batches x 4 heads, n=4096, dh=128) on 8 trn2 cores.

Sharding: one (batch, head) pair per NeuronCore (batch*heads = 8 = n_cores).

v2 strategy (vs baseline):
  - x^T is prepared host-side (fp16, transposed) like W already is: the
    8MB fp32 x load + on-chip convert + transpose become a 4MB fp16 load.
  - V and out^T layout changes go through DMA xbar transposes instead of
    PE transposes + ACT/DVE scatter copies.
  - softmax denominator: running-sum on DVE over exp spans (big-AP adds),
    then a 128-partition reduction via ones-RHS matmuls (N=1, ~free).
  - emission order pipelines groups: proj-q(g+1) and the first S^T span
    of group g+1 are emitted inside group g, and PV chunks are
    interleaved between spans, so the ACT exp stream never stalls at
    group boundaries.
  - PSUM: one 2x3-bank pool for S^T spans + proj accumulators, one
    2x1-bank pool for pv/dn.
"""

import numpy as np
from contextlib import ExitStack

import concourse.bass as bass
import concourse.mybir as mybir
import concourse.tile as tile
from concourse.bass_utils import run_bass_kernel_spmd
from bass_rust import ScopedClock

F32 = mybir.dt.float32
F32R = mybir.dt.float32r
F16 = mybir.dt.float16
AF = mybir.ActivationFunctionType

B = 2
HEADS = 4
N = 4096
DIM = 512
DH = 128
NCORES = 8

SCALE = DH ** -0.5        # folded into the exp activation
EXP_BIAS = -2.0           # exp(s*SCALE - 2): keeps fp16 sums < ~5e3

NG = 8                    # query groups of 512
QG = 512                  # queries per group
KC = 32                   # key chunks of 128
SPAN = 3                  # S^T chunks per exp instruction


def spans():
    out, c = [], 0
    while c < KC:
        out.append((c, min(SPAN, KC - c)))
        c += SPAN
    return out


MAXW = 1  # max sync waits this walrus build accepts per instruction


class _TC(tile.TileContext):
    """TileContext with a post-pass that splits instructions' sem waits
    across preceding same-engine NOPs: this container's walrus rejects any
    instruction carrying more than MAXW sync waits."""

    def _drain_and_barrier(self, tick_clock, wait_clock):
        nc = self.nc
        drain_inst = nc.sync.drain()
        wait_clock.add_sem_waits(
            drain_inst.ins, ScopedClock({None: tick_clock.global_clock})
        )
        nc.all_engine_barrier()
        assert self.sems is not None
        popped = nc._tile_sem_poison_stack.pop()
        assert popped is self._sem_poison
        nc.clear_and_free_semaphores(list(self.sems.allocated().values()))
        nc.all_engine_barrier()
        self._split_excess_waits()

    def _split_excess_waits(self):
        nc = self.nc
        cur_insts = nn_bb_insts(nc)
        for bb in nc.m.functions[0].blocks:
            insts = bb.instructions
            pos = 0
            while pos < len(insts):
                inst = insts[pos]
                si = inst.sync_info
                waits = list(si.on_wait) if si and si.on_wait else []
                if len(waits) <= MAXW:
                    pos += 1
                    continue
                si.on_wait = waits[-MAXW:]
                rest = waits[:-MAXW]
                eng = nc.engines[inst.engine]
                for i in range(0, len(rest), MAXW):
                    chunk = rest[i : i + MAXW]
                    nop = eng.nop()
                    popped = cur_insts.pop()
                    assert popped.name == nop.ins.name
                    nsi = nop.ins.sync_info
                    if nsi is None:
                        nop.ins.sync_info = mybir.SyncInfo(
                            on_wait=chunk, on_update=[]
                        )
                    else:
                        nsi.on_wait = chunk
                    insts.insert(pos, nop.ins)
                    pos += 1
                pos += 1


def nn_bb_insts(nc):
    bb = nc.cur_bb
    assert bb is not None
    return bb.bb.instructions


def build(repeat=1, loop_reps=None):
    nc = bass.Bass()
    # x^T fp16 [DIM, N], host-prepared
    xt = nc.dram_tensor("xt", [DIM, N], F16, kind="ExternalInput")
    # per-head W^T, columns [q | k | v], each [DIM, DH]
    wt = nc.dram_tensor("wt", [DIM, 3 * DH], F32, kind="ExternalInput")
    bqkv = nc.dram_tensor("bqkv", [3, DH], F32, kind="ExternalInput")
    y = nc.dram_tensor("y", [N, DH], F32, kind="ExternalOutput")

    with ExitStack() as ctx:
        tc = ctx.enter_context(_TC(nc))

        singles = ctx.enter_context(tc.tile_pool(name="singles", bufs=1))

        ones16 = singles.tile([128, 1], F16)
        nc.vector.memset(ones16, 1.0)
        expb = singles.tile([128, 1], F32)
        nc.vector.memset(expb, EXP_BIAS)

        # weights [dm-within-chunk, dm-chunk, 3*dh] fp16, biases [dh, 3]
        wt_sb32 = singles.tile([128, 4, 3 * DH], F32)
        nc.sync.dma_start(out=wt_sb32, in_=wt[:, :].rearrange("(c p) o -> p c o", p=128))
        wt16 = singles.tile([128, 4, 3 * DH], F16)
        nc.vector.tensor_copy(out=wt16, in_=wt_sb32)
        b_sb = singles.tile([128, 3], F32)
        nc.sync.dma_start(out=b_sb, in_=bqkv[:, :].rearrange("t d -> d t"))

        # x^T windows: w covers n in [1024*w, 1024*(w+1))
        xtw = [singles.tile([128, 4, 1024], F16, name=f"xtw{w}") for w in range(4)]
        # resident activations
        qd = singles.tile([128, N], F32R)            # Q^T  [dh, n]
        kd = singles.tile([128, N], F32R)            # K^T  [dh, n]
        vsb = singles.tile([128, KC, DH], F16)       # V    [n-in-chunk, chunk, dh]

        if loop_reps is None:
            for _rep in range(repeat):
                _body(nc, tc, ones16, expb, wt16, b_sb, xtw, qd, kd, vsb, xt, y)
        else:
            with tc.For_i(0, loop_reps, 1):
                _body(nc, tc, ones16, expb, wt16, b_sb, xtw, qd, kd, vsb, xt, y)

    return nc


def _body(nc, tc, ones16, expb, wt16, b_sb, xtw, qd, kd, vsb, xt, y):
    ph = ExitStack()
    vtmp = ph.enter_context(tc.tile_pool(name="vtmp", bufs=2))
    pt_pool = ph.enter_context(tc.tile_pool(name="pt", bufs=8))
    acc_pool = ph.enter_context(tc.tile_pool(name="acc", bufs=2))
    cs_pool = ph.enter_context(tc.tile_pool(name="cs", bufs=2))
    ot_pool = ph.enter_context(tc.tile_pool(name="ot", bufs=2))
    oT_pool = ph.enter_context(tc.tile_pool(name="oT", bufs=2))
    ob_pool = ph.enter_context(tc.tile_pool(name="ob", bufs=2))
    rc_pool = ph.enter_context(tc.tile_pool(name="rc", bufs=2))
    # spans [128, 1536] f32 = 3 PSUM banks; same slots serve proj pm tiles
    ps_big = ph.enter_context(tc.tile_pool(name="ps_big", bufs=2, space="PSUM"))
    # pv accumulator + dn rotate through a 2-buf pool (1 bank each)
    ps_sm = ph.enter_context(tc.tile_pool(name="ps_sm", bufs=2, space="PSUM"))

    # ---- load host-transposed x^T (4 windows so proj can start early) ----
    for w in range(4):
        nc.sync.dma_start(
            out=xtw[w],
            in_=xt[:, w * 1024 : (w + 1) * 1024].rearrange("(c p) n -> p c n", p=128),
        )

    def xt_ap(d, nch):
        w, half = divmod(nch, 2)
        return xtw[w][:, d, half * 512 : (half + 1) * 512]

    def proj(m, nch, out_cb):
        pm_t = ps_big.tile([128, 3 * 512], F32, tag="st")
        pm = pm_t[:, 0:512]
        for d in range(4):
            nc.tensor.matmul(
                pm,
                lhsT=wt16[:, d, m * DH : (m + 1) * DH],
                rhs=xt_ap(d, nch),
                start=(d == 0),
                stop=(d == 3),
            )
        out_cb(pm)

    # ---- proj-k and proj-v for all chunks (attention needs full K/V) ----
    for nch in range(8):
        proj(
            1, nch,
            lambda pm, nch=nch: nc.vector.tensor_scalar_add(
                kd[:, nch * 512 : (nch + 1) * 512], pm, b_sb[:, 1:2]
            ),
        )

        def v_out(pm, nch=nch):
            vt = vtmp.tile([128, 512], F16)
            nc.vector.tensor_scalar_add(vt, pm, b_sb[:, 2:3])
            nc.sync.dma_start_transpose(
                out=vsb[:, nch * 4 : (nch + 1) * 4, :], in_=vt
            )

        proj(2, nch, v_out)

    def proj_q(g):
        proj(
            0, g,
            lambda pm: nc.vector.tensor_scalar_add(
                qd[:, g * QG : (g + 1) * QG], pm, b_sb[:, 0:1]
            ),
        )

    proj_q(0)

    SP = spans()
    NSP = len(SP)
    # per-group live state: pts span tiles + acc tile
    pts_of = {}
    acc_of = {}

    def emit_span(g, s):
        q_sl = slice(g * QG, (g + 1) * QG)
        c0, w = SP[s]
        stp = ps_big.tile([128, 3 * 512], F32, tag="st")
        for j in range(w):
            kc = c0 + j
            nc.tensor.matmul(
                stp[:, j * 512 : (j + 1) * 512],
                lhsT=kd[:, kc * 128 : (kc + 1) * 128],
                rhs=qd[:, q_sl],
                start=True,
                stop=True,
            )
        pts = pt_pool.tile([128, 3 * 512], F16, tag="pt")
        nc.scalar.activation(
            out=pts[:, : w * 512],
            in_=stp[:, : w * 512],
            func=AF.Exp,
            scale=SCALE,
            bias=expb,
        )
        pts_of[(g, s)] = pts
        if s == 0:
            acc = acc_pool.tile([128, 3 * 512], F16, tag="acc")
            acc_of[g] = acc
            nc.vector.tensor_copy(out=acc[:, : w * 512], in_=pts[:, : w * 512])
        else:
            acc = acc_of[g]
            nc.vector.tensor_add(
                acc[:, : w * 512], acc[:, : w * 512], pts[:, : w * 512]
            )

    pv_of = {}

    def emit_pv(g, s):
        c0, w = SP[s]
        if s == 0:
            pv_of[g] = ps_sm.tile([128, 512], F32, tag="sm", name=f"pv{g}")
        pv = pv_of[g]
        pts = pts_of.pop((g, s))
        for j in range(w):
            kc = c0 + j
            nc.tensor.matmul(
                pv,
                lhsT=vsb[:, kc, :],
                rhs=pts[:, j * 512 : (j + 1) * 512],
                start=(kc == 0),
                stop=(kc == KC - 1),
            )

    for g in range(NG):
        q_sl = slice(g * QG, (g + 1) * QG)
        if g == 0:
            emit_span(0, 0)
        if g + 1 < NG:
            proj_q(g + 1)
        for s in range(1, NSP):
            emit_span(g, s)
            emit_pv(g, s - 1)
        if g + 1 < NG:
            emit_span(g + 1, 0)
        emit_pv(g, NSP - 1)

        # denominator: fold acc's 3 lanes, then 128-part reduce via ones-RHS
        acc = acc_of.pop(g)
        cs = cs_pool.tile([128, 512], F16)
        nc.vector.tensor_add(cs, acc[:, 0:512], acc[:, 512:1024])
        nc.vector.tensor_add(cs, cs, acc[:, 1024:1536])
        dn = ps_sm.tile([128, 512], F32, tag="sm")
        for st in range(4):
            nc.tensor.matmul(
                dn[:, st : st + 1],
                lhsT=cs[:, st * 128 : (st + 1) * 128],
                rhs=ones16,
                start=True,
                stop=True,
            )
        rc = rc_pool.tile([128, 4], F32)
        nc.vector.reciprocal(rc, dn[:, 0:4])

        # out^T -> fp16 -> xbar transpose -> normalize -> store
        pv = pv_of.pop(g)
        ot16 = ot_pool.tile([128, 512], F16)
        nc.vector.tensor_copy(out=ot16, in_=pv)
        oT = oT_pool.tile([128, 4, DH], F16)
        nc.sync.dma_start_transpose(out=oT, in_=ot16)
        ob = ob_pool.tile([128, 4, DH], F32)
        for st in range(4):
            nc.vector.tensor_scalar_mul(ob[:, st, :], oT[:, st, :], rc[:, st : st + 1])
        nc.sync.dma_start(
            out=y[q_sl, :].rearrange("(s p) d -> p s d", p=128), in_=ob
        )

    ph.close()


def prep_in_maps(x, W, b):
    x = np.asarray(x, dtype=np.float32)
    W = np.asarray(W, dtype=np.float32)
    b = np.asarray(b, dtype=np.float32)
    in_maps = []
    for c in range(NCORES):
        bb, h = divmod(c, HEADS)
        rows = np.arange(DH) * HEADS + h
        wt = np.concatenate(
            [np.ascontiguousarray(W[blk * DIM + rows, :].T) for blk in range(3)],
            axis=1,
        )  # [DIM, 3*DH]
        bs = np.stack([b[blk * DIM + rows] for blk in range(3)], axis=0)  # [3, DH]
        in_maps.append(
            {
                "xt": np.ascontiguousarray(x[bb].T.astype(np.float16)),
                "wt": np.ascontiguousarray(wt),
                "bqkv": np.ascontiguousarray(bs),
            }
        )
    return in_maps


_NC = None


def kernel(x, W, b):
    global _NC
    if _NC is None:
        _NC = build()

    in_maps = prep_in_maps(x, W, b)
    res = run_bass_kernel_spmd(_NC, in_maps, core_ids=list(range(NCORES)))

    out = np.empty((B, N, HEADS * DH), dtype=np.float32)
    for c in range(NCORES):
        bb, h = divmod(c, HEADS)
        out[bb, :, h * DH : (h + 1) * DH] = res.results[c]["y"]
    return out
